# revision 24
# baseline (speedup 1.0000x reference)
"""Dual-score causal attention on 8 Trainium2 NeuronCores.

Math (per batch*head):
    S = (q @ k.T + pe_q @ pe_k.T) * D**-0.5   == concat(q,pe_q) @ concat(k,pe_k).T * scale
    O = softmax(causal_mask(S)) @ v

Sharding: B*H = 32 pairs -> 4 per core (head/data parallel, no collectives).

Layout: all input reshaping is host-side (numpy) so the device only does
linear DMA loads -- no DRAM staging, no on-device DMA transposes:
  - Q' = [q|pe_q], K' = [k|pe_k] arrive pre-transposed d-major [128, L] f16.
  - V arrives as [128, NKB, 65] f16 with a baked ones-column (row-sum trick:
    A@[V|1] yields softmax denominators from the same matmul chain).
  - Output leaves as normalized O [128, NKB, 64] f16; host upcasts to f32.

Per-core compute structure:
  - One global software pipeline over ALL (bh, qi) score stages; no drains
    at qi/bh boundaries.  Stage = up to 3 k-blocks -> S^T tile [128, <=1536]
    in a 3-bank PSUM buffer (x2), exp'd by ONE scalar-engine activation
    (fewer ACTIVATEs amortizes the ~450ns per-instruction overhead; the
    scalar engine is the throughput floor of the whole kernel).
  - Causality: fully-masked k-blocks skipped; diagonal blocks trim dead
    query columns in the matmul; the 128x128 triangle is fixed by a 0/1
    fp16 multiply on VectorE.
  - A@V accumulates O^T [65, 512] per qi in PSUM (row 64 = denominator).
  - Normalization stays in O^T orientation (no PE transposes): O^T is
    copied whole to SBUF f16 (frees the PSUM accumulator after one fast op
    so the next qi's A@V never stalls), the denominator row is replicated
    to the 64 output partitions by a tiny ones-outer-product matmul
    (contraction 1) into the spare PSUM slot, then a fast approximate
    reciprocal (~18-bit, 5x cheaper than exact) and one VectorE multiply.
    The output ships as O^T [64, L]; the host does the final layout
    transpose during unshard.
"""

import os
import sys

import numpy as np

B, H, L, D = 2, 16, 2048, 64
NCORES = 8
BHPC = (B * H) // NCORES  # bh pairs per core = 4
QB = 512  # query block (otp free dim)
KB = 128  # key block (S^T partition dim)
NQB = L // QB  # 4
NKB = L // KB  # 16
KB_PER_QB = QB // KB  # 4
SCALE = float(D) ** -0.5
STW = 1536  # stage width (3 PSUM banks of f32)
# fast-exp on VectorE for offloaded diagonal stages: int16(s*C1 + C2)
# bit-cast as f16 is 2^(s*scale*log2e) with ~3% piecewise-linear error
# (C2's -44 centers the mantissa-interpolation error; softmax ratios
# cancel most of it -- end-to-end worst rel err 2.8e-3 vs 6.5e-4 exact)
FEXP_C1 = SCALE * 1.4426950408889634 * 1024.0
FEXP_C2 = 15.0 * 1024.0 - 44.0
FEXP_QI_MIN = 2  # offload diag stages of qi >= this (deep rows dilute error)

_CACHE = {}


def _import_concourse():
    try:
        import concourse  # noqa: F401
    except ImportError:
        for p in ("/opt/trn_rl_repo", "/root/.axon_site/_ro/trn_rl_repo"):
            if os.path.isdir(p) and p not in sys.path:
                sys.path.insert(0, p)


def _slot(pieces):
    """Assign stage-tile column offsets so no matmul output crosses a 2KB
    PSUM bank boundary (512 f32 columns).  Returns (pieces_with_off, cols)."""
    out = []
    off = 0
    for j, m, n, isdiag in pieces:
        if off // QB != (off + n - 1) // QB:
            off = (off // QB + 1) * QB  # bump to next bank
        out.append((j, m, n, off, isdiag))
        off += n
    return out, off


def _stages_for_qi(qi):
    """Stage list for one query block: each stage is a list of score pieces
    (j, m, n, off, isdiag): k-block j, query-column offset m, column count n,
    stage-tile column offset off.  Full blocks packed 3 per stage; the four
    trimmed diagonal blocks share one stage (bank-aligned slots: 0/512/1024/
    1280, 1408 cols incl. one 128-col dead gap that is exp'd but never read)."""
    nfull = KB_PER_QB * qi
    stages = []
    for j0 in range(0, nfull, 3):
        stages.append(
            _slot([(j, 0, QB, False) for j in range(j0, min(j0 + 3, nfull))])
        )
    stages.append(
        _slot(
            [(nfull + r, KB * r, QB - KB * r, True) for r in (0, 1, 3, 2)]
        )
    )
    return stages


def _build_nc():
    """Build the single-core Bass program (same NEFF for all 8 cores)."""
    _import_concourse()
    from contextlib import ExitStack

    import concourse.tile as tile
    from concourse import bacc, mybir

    f32 = mybir.dt.float32
    f16 = mybir.dt.float16
    i16 = mybir.dt.int16

    nc = bacc.Bacc("TRN2", target_bir_lowering=False, debug=False)

    qT_d = nc.dram_tensor("qT", [BHPC, 128, L], f16, kind="ExternalInput").ap()
    kT_d = nc.dram_tensor("kT", [BHPC, 128, L], f16, kind="ExternalInput").ap()
    v_d = nc.dram_tensor("v", [BHPC, 128, NKB * (D + 1)], f16, kind="ExternalInput").ap()
    tri_d = nc.dram_tensor("tri", [128, 128], f16, kind="ExternalInput").ap()
    ones_d = nc.dram_tensor("ones65", [65, D], f16, kind="ExternalInput").ap()
    out_d = nc.dram_tensor("out", [BHPC, D, L], f16, kind="ExternalOutput").ap()

    Exp = mybir.ActivationFunctionType.Exp

    with tile.TileContext(nc) as tc:
        with ExitStack() as ctx:
            ep = ctx.enter_context

            const_pool = ep(tc.tile_pool(name="const", bufs=1))
            qT_pool = ep(tc.tile_pool(name="qT", bufs=BHPC))
            kT_pool = ep(tc.tile_pool(name="kT", bufs=BHPC))
            v_pool = ep(tc.tile_pool(name="v", bufs=BHPC))
            ex_pool = ep(tc.tile_pool(name="ex", bufs=7))
            ost_pool = ep(tc.tile_pool(name="ost", bufs=2))
            otsb_pool = ep(tc.tile_pool(name="otsb", bufs=2))
            rcinv_pool = ep(tc.tile_pool(name="rcinv", bufs=2))
            stp_pool = ep(tc.tile_pool(name="stp", bufs=2, space="PSUM"))
            otp_pool = ep(tc.tile_pool(name="otp", bufs=2, space="PSUM"))

            tri = const_pool.tile([128, 128], f16)
            ones65 = const_pool.tile([65, D], f16)

            # prefetch every bh's inputs up front (SBUF easily holds all 4):
            # kT on the SP queue, qT+V on the GpSimd queue
            ins = []
            for bh in range(BHPC):
                qT = qT_pool.tile([128, L], f16)
                kT = kT_pool.tile([128, L], f16)
                vsb = v_pool.tile([128, NKB, D + 1], f16)
                if bh == 0:
                    # split loads: the first stage needs only cols [0, 512)
                    nc.sync.dma_start(kT[:, 0:QB], kT_d[bh][:, 0:QB])
                    nc.gpsimd.dma_start(qT[:, 0:QB], qT_d[bh][:, 0:QB])
                    nc.sync.dma_start(tri[:], tri_d)
                    nc.sync.dma_start(ones65[:], ones_d)
                    nc.sync.dma_start(kT[:, QB:L], kT_d[bh][:, QB:L])
                    nc.gpsimd.dma_start(qT[:, QB:L], qT_d[bh][:, QB:L])
                else:
                    nc.sync.dma_start(kT[:], kT_d[bh])
                    nc.gpsimd.dma_start(qT[:], qT_d[bh])
                nc.gpsimd.dma_start(vsb[:], v_d[bh].rearrange("p (n d) -> p n d", n=NKB))
                ins.append((qT, kT, vsb))

            # ---- global stage list over (bh, qi) ----
            # each entry: (bh, qi, pieces, first, last) where first/last flag
            # the qi's PSUM accumulation-group boundaries
            gstages = []
            for bh in range(BHPC):
                for qi in range(NQB):
                    st = _stages_for_qi(qi)
                    npieces = sum(len(s) for s, _ in st)
                    seen = 0
                    for s in st:
                        gstages.append(
                            (bh, qi, s, seen == 0, seen + len(s[0]) == npieces)
                        )
                        seen += len(s[0])

            osts = {}   # bh -> ost tile
            otps = {}   # (bh, qi) -> otp tile
            norm_q = []  # deferred normalize tasks: (bh, qi, otp)

            def emit_s(t):
                bh, qi, (pieces, cols), _, _ = gstages[t]
                qT, kT, _ = ins[bh]
                stp = stp_pool.tile([128, STW], f32, tag="stp")
                for j, m, n, off, _ in pieces:
                    nc.tensor.matmul(
                        stp[:, off : off + n],
                        lhsT=kT[:, j * KB : (j + 1) * KB],
                        rhs=qT[:, qi * QB + m : (qi + 1) * QB],
                        start=True,
                        stop=True,
                        skip_group_check=True,
                    )
                if pieces[0][4] and qi >= FEXP_QI_MIN:
                    # VectorE fast-exp (offloads the saturated scalar engine)
                    exi = ex_pool.tile([128, STW], i16, tag="ex", name="ex")
                    nc.vector.tensor_scalar(
                        out=exi[:, 0:cols],
                        in0=stp[:, 0:cols],
                        scalar1=FEXP_C1,
                        scalar2=FEXP_C2,
                        op0=mybir.AluOpType.mult,
                        op1=mybir.AluOpType.add,
                    )
                    ex = exi[:].bitcast(f16)
                else:
                    ex = ex_pool.tile([128, STW], f16, tag="ex", name="ex")
                    nc.scalar.activation(
                        ex[:, 0:cols], stp[:, 0:cols], Exp, scale=SCALE
                    )
                # triangle fix on each diagonal piece's leading 128 cols
                for j, m, n, off, isdiag in pieces:
                    if isdiag:
                        nc.vector.tensor_mul(
                            ex[:, off : off + KB], ex[:, off : off + KB], tri[:]
                        )
                return ex

            def emit_av(t, ex):
                bh, qi, (pieces, _), first, last = gstages[t]
                _, _, vsb = ins[bh]
                key = (bh, qi)
                if key not in otps:
                    otps[key] = otp_pool.tile(
                        [D + 1, QB], f32, tag="otp", name="otp"
                    )
                otp = otps[key]
                for i, (j, m, n, off, _) in enumerate(pieces):
                    nc.tensor.matmul(
                        otp[:, m:QB],
                        lhsT=vsb[:, j, :],
                        rhs=ex[:, off : off + n],
                        start=first and i == 0,
                        stop=last and i == len(pieces) - 1,
                        skip_group_check=True,
                    )
                if last:
                    norm_q.append((bh, qi, otps.pop(key)))

            def emit_norm():
                bh, qi, otp = norm_q.pop(0)
                if bh not in osts:
                    osts[bh] = ost_pool.tile([D, NQB, QB], f16, name="ost")
                ost = osts[bh]
                # O^T -> SBUF f16 in one op; otp's ring slot frees right here
                otsb = otsb_pool.tile([D + 1, QB], f16, name="otsb")
                nc.vector.tensor_copy(otsb[:], otp[:])
                # replicate denominator row to 64 partitions: ones[64] outer
                # denom-row (contraction-1 f16 matmul, spare otp-ring slot)
                rcb = otp_pool.tile([D, QB], f32, tag="otp", name="rcb")
                nc.tensor.matmul(
                    rcb[:],
                    lhsT=ones65[D : D + 1, :],
                    rhs=otsb[D : D + 1, :],
                    start=True,
                    stop=True,
                    skip_group_check=True,
                )
                rcinv = rcinv_pool.tile([D, QB], f32, name="rcinv")
                nc.vector.reciprocal_approx_fast(out=rcinv[:], in_=rcb[:])
                nc.vector.tensor_mul(ost[:, qi, :], otsb[0:D, :], rcinv[:])
                nc.gpsimd.dma_start(
                    out_d[bh][:, qi * QB : (qi + 1) * QB], ost[:, qi, :]
                )
                if qi == NQB - 1:
                    osts.pop(bh)

            # ---- the pump: one software pipeline across everything ----
            LAG = 4
            nst = len(gstages)
            exs = {}
            for t in range(nst + LAG):
                if t < nst:
                    exs[t] = emit_s(t)
                # deferred normalize: emitted between S matmuls and the lagged
                # AV so its PSUM->SBUF copy dependency is already satisfied
                if norm_q:
                    emit_norm()
                if t >= LAG:
                    emit_av(t - LAG, exs.pop(t - LAG))
            while norm_q:
                emit_norm()

    nc.compile()
    return nc


def _host_consts():
    kk = np.arange(128)[:, None]
    cc = np.arange(128)[None, :]
    tri = (kk <= cc).astype(np.float16)
    ones65 = np.ones((65, D), dtype=np.float16)
    return tri, ones65


def _shard_inputs(q, k, v, pe_q, pe_k):
    q = np.asarray(q, dtype=np.float32).reshape(B * H, L, D)
    k = np.asarray(k, dtype=np.float32).reshape(B * H, L, D)
    v = np.asarray(v, dtype=np.float32).reshape(B * H, L, D)
    pe_q = np.asarray(pe_q, dtype=np.float32).reshape(B * H, L, D)
    pe_k = np.asarray(pe_k, dtype=np.float32).reshape(B * H, L, D)
    # host-side layout packing (no math): d-major f16 Q'/K', V with baked
    # ones column; the device then only does linear DMA loads
    qT = np.ascontiguousarray(
        np.concatenate([q, pe_q], axis=-1).transpose(0, 2, 1)
    ).astype(np.float16)  # [B*H, 128, L]
    kT = np.ascontiguousarray(
        np.concatenate([k, pe_k], axis=-1).transpose(0, 2, 1)
    ).astype(np.float16)  # [B*H, 128, L]
    vp = v.reshape(B * H, NKB, 128, D).transpose(0, 2, 1, 3)  # [B*H, 128, NKB, D]
    vsb = np.empty((B * H, 128, NKB, D + 1), dtype=np.float16)
    vsb[..., 0:D] = vp
    vsb[..., D] = 1.0
    vsb = vsb.reshape(B * H, 128, NKB * (D + 1))
    tri, ones65 = _host_consts()
    in_maps = []
    for c in range(NCORES):
        s = slice(c * BHPC, (c + 1) * BHPC)
        in_maps.append(
            {
                "qT": np.ascontiguousarray(qT[s]),
                "kT": np.ascontiguousarray(kT[s]),
                "v": np.ascontiguousarray(vsb[s]),
                "tri": tri,
                "ones65": ones65,
            }
        )
    return in_maps


def kernel(q, k, v, pe_q, pe_k, mask=None, **_ignored):
    """Full-input entry point: shards across 8 NeuronCores, returns full output.

    The mask input is the (fixed) causal mask of the problem; causality is
    implemented structurally in the device kernel, so it is not shipped.
    """
    _import_concourse()
    from concourse.bass_utils import run_bass_kernel_spmd

    if "nc" not in _CACHE:
        _CACHE["nc"] = _build_nc()
    nc = _CACHE["nc"]

    in_maps = _shard_inputs(q, k, v, pe_q, pe_k)
    res = run_bass_kernel_spmd(nc, in_maps, core_ids=list(range(NCORES)))
    out = np.empty((B * H, L, D), dtype=np.float32)
    for c in range(NCORES):
        # device output is O^T [bh, D, L]; final layout transpose on host
        out[c * BHPC : (c + 1) * BHPC] = (
            res.results[c]["out"].astype(np.float32).transpose(0, 2, 1)
        )
    return out.reshape(B, H, L, D)
